# revision 18
# baseline (speedup 1.0000x reference)
"""GQA attention prefill kernel for 8 Trainium2 NeuronCores.

Sharding: data-parallel over batch (2) x tensor-parallel over kv-head
groups (4 groups of 2 kv-heads + their 8 q-heads). Each core computes
its partial out = attn_shard @ wo_shard; the host sums the 4
row-parallel partials per batch.

v2 design (vs the fp32r baseline):
- Everything on-chip is fp16 (full PE rate, half the DMA bytes, 2x DVE
  modes, ~8x less rounding error than bf16). PSUM accumulation stays
  fp32.
- x is pre-transposed on the host into k-tile-major [128, 32, 1024]
  layout, so the on-chip transpose phase (256 PE transposes + PSUM
  evictions) disappears.
- Weights are pre-permuted on the host into the exact SBUF slab layout
  ([128 part, out-tile, k-tile, 128]), so every weight DMA is a
  contiguous multi-KB line per partition and each weight byte is
  loaded exactly once.
- Attention (scores -> exp -> softmax -> PV) for head h is interleaved
  under the Q projection of head h+1, so the scalar-engine exp and the
  DVE softmax work hide entirely under projection matmuls.
- RoPE is applied in [head_dim, tokens] layout via host-permuted
  even/odd weight columns (rotate-half becomes partition-half ops).
- Output partials are stored as fp16 [4096, 1024]; the host upcasts,
  sums the 4 group partials and transposes.

Relies on harness input semantics: mask is all zeros and input_indexes
is arange(S) (the kv cache is exactly the freshly projected K/V), as
fixed by the problem's input_specs.
"""
import numpy as np
from contextlib import ExitStack

import concourse.bass as bass
import concourse.tile as tile
from concourse import bacc, mybir
from concourse.bass_utils import run_bass_kernel_spmd
from concourse.masks import make_identity

dt = mybir.dt

DIM = 4096
N_HEADS = 32
N_KV = 8
HD = 128
B = 2
S = 1024
NCORES = 8
HPC = 8    # q-heads per core
KVPC = 2   # kv-heads per core
P = 128
HALF = 512
NKT = DIM // P      # 32 k-tiles over DIM
NTT = S // P        # 8 token tiles
NOT = DIM // P      # 32 out tiles (phase D)
SCALE = 1.0 / np.sqrt(HD)

_CACHE = {}


def _build():
    nc = bacc.Bacc("TRN2", target_bir_lowering=False, debug=False,
                   num_devices=NCORES)
    xt_d = nc.dram_tensor("xt", [P, NKT, S], dt.float16, kind="ExternalInput").ap()
    wq_d = nc.dram_tensor("wq", [P, HPC, NKT, P], dt.float16, kind="ExternalInput").ap()
    wk_d = nc.dram_tensor("wk", [P, KVPC, NKT, P], dt.float16, kind="ExternalInput").ap()
    wv_d = nc.dram_tensor("wv", [P, KVPC, NKT, P], dt.float16, kind="ExternalInput").ap()
    wo_d = nc.dram_tensor("wo", [P, NOT, HPC, P], dt.float16, kind="ExternalInput").ap()
    cos_d = nc.dram_tensor("cos2", [P, S], dt.float16, kind="ExternalInput").ap()
    sin_d = nc.dram_tensor("sinpm", [P, S], dt.float16, kind="ExternalInput").ap()
    out_d = nc.dram_tensor("out", [DIM, S], dt.float16, kind="ExternalOutput").ap()

    with tile.TileContext(nc) as tc:
        with ExitStack() as ctx:
            persist = ctx.enter_context(tc.tile_pool(name="persist", bufs=1))
            # PSUM budget (8 banks): sc 2x2 + pj 2x1 + po 2x1 = 8.
            psA = ctx.enter_context(tc.tile_pool(name="psA", bufs=2, space="PSUM"))
            psB = ctx.enter_context(tc.tile_pool(name="psB", bufs=2, space="PSUM"))
            psC = ctx.enter_context(tc.tile_pool(name="psC", bufs=2, space="PSUM"))
            wqp = ctx.enter_context(tc.tile_pool(name="wqp", bufs=4))
            wop = ctx.enter_context(tc.tile_pool(name="wop", bufs=4))
            rp = ctx.enter_context(tc.tile_pool(name="rp", bufs=2))
            ep = ctx.enter_context(tc.tile_pool(name="ep", bufs=2))
            trp = ctx.enter_context(tc.tile_pool(name="trp", bufs=1))
            srp = ctx.enter_context(tc.tile_pool(name="srp", bufs=1))
            op = ctx.enter_context(tc.tile_pool(name="op", bufs=4))

            ident = persist.tile([P, P], dt.float32, tag="ident")
            make_identity(nc, ident[:])
            ident_h = persist.tile([P, P], dt.float16, tag="ident_h")
            nc.scalar.copy(ident_h[:], ident[:])
            ones_f = persist.tile([P, 1], dt.float32, tag="ones_f")
            nc.gpsimd.memset(ones_f[:], 1.0)
            ones_h = persist.tile([P, 1], dt.float16, tag="ones_h")
            nc.scalar.copy(ones_h[:], ones_f[:])
            onesr_f = persist.tile([1, P], dt.float32, tag="onesr_f")
            nc.gpsimd.memset(onesr_f[:], 1.0)
            ones_r = persist.tile([1, P], dt.float16, tag="ones_r")
            nc.scalar.copy(ones_r[:], onesr_f[:])

            # PE warmup (HAM) while the first DMAs land; also preload the
            # Exp activation table.
            for i in range(30):
                warm = psA.tile([P, P], dt.float16, tag="sc", name=f"warm{i}")
                nc.tensor.transpose(warm[:], ident_h[:], ident_h[:])
            dummy = rp.tile([P, S], dt.float16, tag="ev", name="expwarm")
            nc.scalar.activation(dummy[:, 0:P], ident_h[:],
                                 mybir.ActivationFunctionType.Exp, scale=1.0)

            # cos/sin ride the scalar engine's DMA queue so they don't wait
            # behind the bulk x/weight stream on the sync queue.
            cos2 = persist.tile([P, S], dt.float16, tag="cos2")
            nc.scalar.dma_start(cos2[:], cos_d[:])
            sinpm = persist.tile([P, S], dt.float16, tag="sinpm")
            nc.scalar.dma_start(sinpm[:], sin_d[:])

            kt_t = persist.tile([P, KVPC, S], dt.float16, tag="kt")
            vnat = persist.tile([P, NTT, KVPC * HD], dt.float16, tag="vnat")
            qt = [persist.tile([P, S], dt.float16, tag=f"qa{h}", name=f"qt{h}")
                  for h in range(HPC)]

            def slab(w_dram, idx, nm):
                t = wqp.tile([P, NKT, P], dt.float16, tag="w", name=nm)
                nc.sync.dma_start(t[:], w_dram[:, idx])
                return t

            # DMA order on the sync queue matters: K/V slab chunks are
            # interleaved with the x stream in k order so the single-pass
            # B-KV k-loop can start as soon as the first k-tiles land.
            sk0 = wqp.tile([P, NKT, P], dt.float16, tag="w", name="sk0")
            sv0 = wqp.tile([P, NKT, P], dt.float16, tag="w", name="sv0")
            sk1 = wqp.tile([P, NKT, P], dt.float16, tag="w", name="sk1")
            sv1 = wqp.tile([P, NKT, P], dt.float16, tag="w", name="sv1")
            xT = persist.tile([P, NKT, S], dt.float16, tag="xT")
            # first group in 2-k pieces so the very first matmuls can start
            # ~2us in; later groups in 8-k pieces
            groups = [slice(2 * i, 2 * (i + 1)) for i in range(4)] + \
                     [slice(8 * g, 8 * (g + 1)) for g in range(1, 4)]
            for ks in groups:
                nc.sync.dma_start(sk0[:, ks, :], wk_d[:, 0, ks, :])
                nc.sync.dma_start(sv0[:, ks, :], wv_d[:, 0, ks, :])
                nc.sync.dma_start(sk1[:, ks, :], wk_d[:, 1, ks, :])
                nc.sync.dma_start(sv1[:, ks, :], wv_d[:, 1, ks, :])
                for k in range(ks.start, ks.stop):
                    nc.sync.dma_start(xT[:, k, :], xt_d[:, k, :])

            def rope_evict(pq0, pq1, dest_ap):
                # psum fp32 -> fp16, then rotate-half rope in fp16 on DVE
                ev = rp.tile([P, S], dt.float16, tag="ev")
                nc.scalar.copy(ev[:, 0:HALF], pq0[:])
                nc.scalar.copy(ev[:, HALF:S], pq1[:])
                t1 = rp.tile([P, S], dt.float16, tag="t1")
                t2 = rp.tile([P, S], dt.float16, tag="t2")
                # sinsw rows: [0:64]=+sin (pairs ev re rows), [64:128]=-sin
                # (pairs ev im rows) so each DVE mul reads matching base
                # partitions (tensor_tensor SBUF inputs must share base).
                nc.vector.tensor_mul(out=t1[:], in0=ev[:], in1=cos2[:])
                nc.vector.tensor_mul(out=t2[0:64, :], in0=ev[64:P, :],
                                     in1=sinpm[64:P, :])
                nc.vector.tensor_mul(out=t2[64:P, :], in0=ev[0:64, :],
                                     in1=sinpm[0:64, :])
                nc.vector.tensor_add(out=dest_ap, in0=t1[:], in1=t2[:])

            # ---- Phase B-KV: all four K/V out-tiles in one k-outer pass ----
            # (8 MMs per k-tile keeps the PE ahead of the x DMA stream;
            # uses all 8 PSUM banks: K0->psB, K1->psC, V0/V1->psA.)
            pk0 = psB.tile([P, HALF], dt.float32, tag="pj", name="pk0")
            pk1 = psB.tile([P, HALF], dt.float32, tag="pj", name="pk1")
            pk2 = psC.tile([P, HALF], dt.float32, tag="po", name="pk2")
            pk3 = psC.tile([P, HALF], dt.float32, tag="po", name="pk3")
            pvA = psA.tile([P, S], dt.float32, tag="sc", name="pvA")
            pvB = psA.tile([P, S], dt.float32, tag="sc", name="pvB")
            for k in range(NKT):
                st, sp = (k == 0), (k == NKT - 1)
                x0, x1 = xT[:, k, 0:HALF], xT[:, k, HALF:S]
                nc.tensor.matmul(pk0[:], sk0[:, k], x0, start=st, stop=sp)
                nc.tensor.matmul(pk1[:], sk0[:, k], x1, start=st, stop=sp)
                nc.tensor.matmul(pk2[:], sk1[:, k], x0, start=st, stop=sp)
                nc.tensor.matmul(pk3[:], sk1[:, k], x1, start=st, stop=sp)
                nc.tensor.matmul(pvA[:, 0:HALF], sv0[:, k], x0,
                                 start=st, stop=sp, skip_group_check=True)
                nc.tensor.matmul(pvA[:, HALF:S], sv0[:, k], x1,
                                 start=st, stop=sp, skip_group_check=True)
                nc.tensor.matmul(pvB[:, 0:HALF], sv1[:, k], x0,
                                 start=st, stop=sp, skip_group_check=True)
                nc.tensor.matmul(pvB[:, HALF:S], sv1[:, k], x1,
                                 start=st, stop=sp, skip_group_check=True)
            rope_evict(pk0, pk1, kt_t[:, 0, :])
            rope_evict(pk2, pk3, kt_t[:, 1, :])

            # ---- Q0 projection (V evict/transposes queued behind it) ----
            sq = slab(wq_d, 0, "sq0")
            pq0 = psB.tile([P, HALF], dt.float32, tag="pj", name="pq0_0")
            pq1 = psB.tile([P, HALF], dt.float32, tag="pj", name="pq1_0")
            for k in range(NKT):
                st, sp = (k == 0), (k == NKT - 1)
                nc.tensor.matmul(pq0[:], sq[:, k], xT[:, k, 0:HALF], start=st, stop=sp)
                nc.tensor.matmul(pq1[:], sq[:, k], xT[:, k, HALF:S], start=st, stop=sp)
            rope_evict(pq0, pq1, qt[0][:])
            vteA = rp.tile([P, S], dt.float16, tag="ev", name="vteA")
            nc.scalar.copy(vteA[:], pvA[:])
            vteB = rp.tile([P, S], dt.float16, tag="ev", name="vteB")
            nc.scalar.copy(vteB[:], pvB[:])
            for kv in range(KVPC):
                vte = vteA if kv == 0 else vteB
                for tt in range(NTT):
                    ptv = psC.tile([P, P], dt.float16, tag="po",
                                   name=f"tv{kv}_{tt}")
                    nc.tensor.transpose(ptv[:], vte[:, tt * P:(tt + 1) * P],
                                        ident_h[:])
                    nc.vector.tensor_copy(vnat[:, tt, kv * HD:(kv + 1) * HD],
                                          ptv[:])
            sq = slab(wq_d, 1, "sq1")

            # ---- Stage 2: per-head attention interleaved with Q proj h+1 ----
            for h in range(HPC):
                kv = h // 4
                e = ep.tile([P, NTT, S], dt.float16, tag="e", name=f"e{h}")
                if h < HPC - 1:
                    pq0 = psB.tile([P, HALF], dt.float32, tag="pj",
                                   name=f"pq0_{h+1}")
                    pq1 = psB.tile([P, HALF], dt.float32, tag="pj",
                                   name=f"pq1_{h+1}")
                tree = [None] * 4
                po0 = psC.tile([P, HALF], dt.float32, tag="po", name=f"po0_{h}")
                po1 = psC.tile([P, HALF], dt.float32, tag="po", name=f"po1_{h}")
                last = (h == HPC - 1)

                def pv_step(tt):
                    vtile = vnat[:, tt, kv * HD:(kv + 1) * HD]
                    st, sp = (tt == 0), (tt == NTT - 1)
                    nc.tensor.matmul(po0[:], vtile, e[:, tt, 0:HALF],
                                     start=st, stop=sp)
                    nc.tensor.matmul(po1[:], vtile, e[:, tt, HALF:S],
                                     start=st, stop=sp)

                for tt in range(NTT):
                    sc = psA.tile([P, S], dt.float32, tag="sc",
                                  name=f"sc{h}_{tt}")
                    ktile = kt_t[:, kv, tt * P:(tt + 1) * P]
                    nc.tensor.matmul(sc[:, 0:HALF], ktile, qt[h][:, 0:HALF],
                                     start=True, stop=True)
                    nc.tensor.matmul(sc[:, HALF:S], ktile, qt[h][:, HALF:S],
                                     start=True, stop=True)
                    nc.scalar.activation(e[:, tt, :], sc[:],
                                         mybir.ActivationFunctionType.Exp,
                                         scale=float(SCALE))
                    if not last:
                        for j in range(4):
                            k = tt * 4 + j
                            st, sp = (k == 0), (k == NKT - 1)
                            nc.tensor.matmul(pq0[:], sq[:, k], xT[:, k, 0:HALF],
                                             start=st, stop=sp)
                            nc.tensor.matmul(pq1[:], sq[:, k], xT[:, k, HALF:S],
                                             start=st, stop=sp)
                    elif tt >= 1:
                        # no proj filler for the last head: stagger PV one
                        # tile behind scores to cover the exp latency
                        pv_step(tt - 1)
                    # partial softmax-denominator tree on DVE (fp16, 2x mode);
                    # merge early so little tree work remains after the last exp
                    if tt % 2 == 1:
                        i = tt // 2
                        tree[i] = trp.tile([P, S], dt.float16, tag=f"s{i}",
                                           name=f"tr{h}_{i}")
                        nc.vector.tensor_add(out=tree[i][:], in0=e[:, tt - 1, :],
                                             in1=e[:, tt, :])
                    if tt == 3:
                        nc.vector.tensor_add(out=tree[0][:], in0=tree[0][:],
                                             in1=tree[1][:])
                if not last:
                    rope_evict(pq0, pq1, qt[h + 1][:])
                nc.vector.tensor_add(out=tree[2][:], in0=tree[2][:],
                                     in1=tree[3][:])
                nc.vector.tensor_add(out=tree[0][:], in0=tree[0][:],
                                     in1=tree[2][:])
                if not last:
                    for tt in range(NTT):
                        pv_step(tt)
                else:
                    pv_step(NTT - 1)
                # denominator: sum over partitions via ones-matmul
                pss0 = psA.tile([1, HALF], dt.float32, tag="sc",
                                name=f"pss0_{h}")
                nc.tensor.matmul(pss0[:], ones_h[:], tree[0][:, 0:HALF],
                                 start=True, stop=True)
                pss1 = psA.tile([1, HALF], dt.float32, tag="sc",
                                name=f"pss1_{h}")
                nc.tensor.matmul(pss1[:], ones_h[:], tree[0][:, HALF:S],
                                 start=True, stop=True)
                if not last:
                    # recip + broadcast hide under the next head's work
                    srow = srp.tile([1, S], dt.float32, tag="sr",
                                    name=f"srow{h}")
                    nc.scalar.copy(srow[:, 0:HALF], pss0[:])
                    nc.scalar.copy(srow[:, HALF:S], pss1[:])
                    rci = srp.tile([1, S], dt.float32, tag="rc", name=f"rci{h}")
                    nc.vector.reciprocal_approx_fast(rci[:], srow[:])
                    rcb = srp.tile([P, S], dt.float32, tag="rb", name=f"rcb{h}")
                    nc.gpsimd.partition_broadcast(rcb[:], rci[:])
                else:
                    # last head: nothing left to hide behind, so keep the
                    # normalization chain short — PE-broadcast the denominator
                    # (ones_r outer product) instead of the slow gpsimd
                    # partition_broadcast, then reciprocal on all partitions.
                    srow_h = srp.tile([1, S], dt.float16, tag="sr",
                                      name="srow7h")
                    nc.scalar.copy(srow_h[:, 0:HALF], pss0[:])
                    nc.scalar.copy(srow_h[:, HALF:S], pss1[:])
                    db0 = psA.tile([P, HALF], dt.float32, tag="sc", name="db0")
                    nc.tensor.matmul(db0[:], ones_r[:], srow_h[:, 0:HALF],
                                     start=True, stop=True)
                    db1 = psA.tile([P, HALF], dt.float32, tag="sc", name="db1")
                    nc.tensor.matmul(db1[:], ones_r[:], srow_h[:, HALF:S],
                                     start=True, stop=True)
                    den = srp.tile([P, S], dt.float32, tag="rb", name="den7")
                    nc.scalar.copy(den[:, 0:HALF], db0[:])
                    nc.scalar.copy(den[:, HALF:S], db1[:])
                    rcb = srp.tile([P, S], dt.float32, tag="rc", name="rcb7")
                    nc.vector.reciprocal_approx_fast(rcb[:], den[:])
                attn = persist.tile([P, S], dt.float16, tag=f"qa{h}",
                                    name=f"attn{h}")
                nc.vector.tensor_mul(out=attn[:, 0:HALF], in0=po0[:],
                                     in1=rcb[:, 0:HALF])
                nc.vector.tensor_mul(out=attn[:, HALF:S], in0=po1[:],
                                     in1=rcb[:, HALF:S])
                qt[h] = attn
                if h < HPC - 2:
                    sq = slab(wq_d, h + 2, f"sq{h+2}")

            # ---- Phase D: out projection ----
            for ot in range(NOT):
                wosb = wop.tile([P, HPC, P], dt.float16, tag="wo",
                                name=f"wo{ot}")
                nc.sync.dma_start(wosb[:], wo_d[:, ot])
                pool = psB if ot % 2 == 0 else psC
                tag = "pj" if ot % 2 == 0 else "po"
                pd0 = pool.tile([P, HALF], dt.float32, tag=tag, name=f"pd0_{ot}")
                pd1 = pool.tile([P, HALF], dt.float32, tag=tag, name=f"pd1_{ot}")
                for ht in range(HPC):
                    st, sp = (ht == 0), (ht == HPC - 1)
                    nc.tensor.matmul(pd0[:], wosb[:, ht, :], qt[ht][:, 0:HALF],
                                     start=st, stop=sp)
                    nc.tensor.matmul(pd1[:], wosb[:, ht, :], qt[ht][:, HALF:S],
                                     start=st, stop=sp)
                o0 = op.tile([P, HALF], dt.float16, tag="o", name=f"o0_{ot}")
                nc.scalar.copy(o0[:], pd0[:])
                nc.sync.dma_start(out_d[ot * P:(ot + 1) * P, 0:HALF], o0[:])
                o1 = op.tile([P, HALF], dt.float16, tag="o", name=f"o1_{ot}")
                nc.vector.tensor_copy(o1[:], pd1[:])
                nc.sync.dma_start(out_d[ot * P:(ot + 1) * P, HALF:S], o1[:])

    nc.compile()
    return nc


def _get_nc():
    if "nc" not in _CACHE:
        _CACHE["nc"] = _build()
    return _CACHE["nc"]


def _host_prep(x, freqs_cos, freqs_sin, wq, wk, wv, wo):
    x = np.asarray(x, dtype=np.float32)
    wq = np.asarray(wq, dtype=np.float32)
    wk = np.asarray(wk, dtype=np.float32)
    wv = np.asarray(wv, dtype=np.float32)
    wo = np.asarray(wo, dtype=np.float32)
    perm = np.empty(HD, np.int64)
    perm[0:64] = 2 * np.arange(64)
    perm[64:HD] = 2 * np.arange(64) + 1
    wqp = wq.reshape(DIM, N_HEADS, HD)[:, :, perm]
    wkp = wk.reshape(DIM, N_KV, HD)[:, :, perm]
    wvr = wv.reshape(DIM, N_KV, HD)
    cosT = np.asarray(freqs_cos, np.float32).T  # [64, S]
    sinT = np.asarray(freqs_sin, np.float32).T
    cos2 = np.ascontiguousarray(
        np.concatenate([cosT, cosT], axis=0)).astype(np.float16)   # [128, S]
    # swapped-half layout: rows 0..63 = +sin (multiplies ev re rows via
    # t2[64:128]), rows 64..127 = -sin (multiplies ev im rows via t2[0:64])
    sinpm = np.ascontiguousarray(
        np.concatenate([sinT, -sinT], axis=0)).astype(np.float16)

    def wslab(w_c, nh):
        # [DIM, nh, HD] -> [P, nh(out-tile), NKT, P]
        return np.ascontiguousarray(
            w_c.reshape(NKT, P, nh, HD).transpose(1, 2, 0, 3)).astype(np.float16)

    in_maps = []
    xt_b = {}
    for b in range(B):
        # [S, DIM] -> [P, NKT, S]
        xt_b[b] = np.ascontiguousarray(
            x[b].reshape(S, NKT, P).transpose(2, 1, 0)).astype(np.float16)
    for core in range(NCORES):
        b, g = core // 4, core % 4
        wo_c = wo[HPC * HD * g: HPC * HD * (g + 1), :]  # [1024, DIM]
        wo_slab = np.ascontiguousarray(
            wo_c.reshape(HPC, P, NOT, P).transpose(1, 2, 0, 3)).astype(np.float16)
        in_maps.append({
            "xt": xt_b[b],
            "wq": wslab(wqp[:, HPC * g: HPC * (g + 1), :], HPC),
            "wk": wslab(wkp[:, KVPC * g: KVPC * (g + 1), :], KVPC),
            "wv": wslab(wvr[:, KVPC * g: KVPC * (g + 1), :], KVPC),
            "wo": wo_slab,
            "cos2": cos2,
            "sinpm": sinpm,
        })
    return in_maps


def kernel(x, freqs_cos, freqs_sin, mask, input_indexes, wq, wk, wv, wo,
           cache_k, cache_v, **_ignored):
    in_maps = _host_prep(x, freqs_cos, freqs_sin, wq, wk, wv, wo)
    nc = _get_nc()
    res = run_bass_kernel_spmd(nc, in_maps, core_ids=list(range(NCORES)))
    outs = [res.results[c]["out"] for c in range(NCORES)]
    out = np.empty((B, S, DIM), np.float32)
    for b in range(B):
        acc = outs[4 * b].astype(np.float32)
        for g in range(1, 4):
            acc = acc + outs[4 * b + g].astype(np.float32)
        out[b] = acc.T
    return out


# revision 32
# speedup vs baseline: 1.0559x; 1.0559x over previous
"""GQA attention prefill kernel for 8 Trainium2 NeuronCores.

Sharding: data-parallel over batch (2) x tensor-parallel over kv-head
groups (4 groups of 2 kv-heads + their 8 q-heads). Each core computes
its partial out = attn_shard @ wo_shard; the host sums the 4
row-parallel partials per batch.

Design (measured ~374us vs the 553us fp32r baseline; PE busy ~351us of
a ~335us streaming floor):
- Everything on-chip is fp16 (full PE rate, half the DMA bytes, 2x DVE
  modes, ~8x less rounding error than bf16). PSUM accumulation stays
  fp32.
- x is pre-transposed on the host into k-tile-major [128, 32, 1024]
  layout, so the on-chip transpose phase (256 PE transposes + PSUM
  evictions) disappears.
- Weights are pre-permuted on the host into the exact SBUF slab layout
  ([128 part, out-tile, k-tile, 128]), so every weight DMA is a
  contiguous multi-KB line per partition and each weight byte is
  loaded exactly once.
- Phase B runs as two k-outer passes sized so the PE stays ahead of
  the x/weight DMA stream (pass 1: K0+V0+V1 across 6 PSUM banks with
  V slabs on the scalar engine's parallel DMA queue; pass 2: K1
  staggered with Q0 on a 4-k offset) and each pass's PSUM evictions
  hide under the next pass (a naive single 8-bank pass serializes all
  four evict+rope chains at the boundary and deadlock-couples the
  ACT queue order with PSUM buffer reuse).
- Attention for head h is interleaved under the Q projection of head
  h+1 per key-tile, so the scalar-engine exp (ACT is the pacing
  engine within attention) and the DVE softmax hide under projection
  matmuls; the softmax denominator is a running fp16 DVE sum folded
  by a [128,1]-ones matmul, with recip+partition_broadcast hidden
  under the next head. The last head (no projection to hide under)
  staggers PV one tile behind scores and swaps the slow gpsimd
  partition_broadcast for a [1,128]-ones PE broadcast + reciprocal
  straight from PSUM; phase D's first two out-tiles split their
  8-head accumulation at head 7 so they need not wait for its
  normalization.
- RoPE is applied in [head_dim, tokens] layout via host-permuted
  even/odd weight columns (rotate-half becomes partition-half ops on
  matching base partitions).
- Output partials are stored as fp16 [4096, 1024]; the host upcasts,
  sums the 4 group partials and transposes.

Relies on harness input semantics: mask is all zeros and input_indexes
is arange(S) (the kv cache is exactly the freshly projected K/V), as
fixed by the problem's input_specs.
"""
import numpy as np
from contextlib import ExitStack

import concourse.bass as bass
import concourse.tile as tile
from concourse import bacc, mybir
from concourse import bass_isa
from concourse.bass_utils import run_bass_kernel_spmd

dt = mybir.dt

DIM = 4096
N_HEADS = 32
N_KV = 8
HD = 128
B = 2
S = 1024
NCORES = 8
HPC = 8    # q-heads per core
KVPC = 2   # kv-heads per core
P = 128
HALF = 512
NKT = DIM // P      # 32 k-tiles over DIM
NTT = S // P        # 8 token tiles
NOT = DIM // P      # 32 out tiles (phase D)
SCALE = 1.0 / np.sqrt(HD)

_CACHE = {}


def _build():
    nc = bacc.Bacc("TRN2", target_bir_lowering=False, debug=False,
                   num_devices=NCORES)
    xt_d = nc.dram_tensor("xt", [P, NKT, S], dt.float16, kind="ExternalInput").ap()
    wq_d = nc.dram_tensor("wq", [P, HPC, NKT, P], dt.float16, kind="ExternalInput").ap()
    wk_d = nc.dram_tensor("wk", [P, KVPC, NKT, P], dt.float16, kind="ExternalInput").ap()
    wv_d = nc.dram_tensor("wv", [P, KVPC, NKT, P], dt.float16, kind="ExternalInput").ap()
    wo_d = nc.dram_tensor("wo", [P, NOT, HPC, P], dt.float16, kind="ExternalInput").ap()
    id_d = nc.dram_tensor("identh", [P, P], dt.float16, kind="ExternalInput").ap()
    cos_d = nc.dram_tensor("cos2", [P, S], dt.float16, kind="ExternalInput").ap()
    sin_d = nc.dram_tensor("sinpm", [P, S], dt.float16, kind="ExternalInput").ap()
    out_d = nc.dram_tensor("out", [DIM, S], dt.float16, kind="ExternalOutput").ap()

    with tile.TileContext(nc) as tc:
        with ExitStack() as ctx:
            persist = ctx.enter_context(tc.tile_pool(name="persist", bufs=1))
            # PSUM budget (8 banks): sc 2x2 + pj 2x1 + po 2x1 = 8.
            psA = ctx.enter_context(tc.tile_pool(name="psA", bufs=2, space="PSUM"))
            psB = ctx.enter_context(tc.tile_pool(name="psB", bufs=2, space="PSUM"))
            psC = ctx.enter_context(tc.tile_pool(name="psC", bufs=2, space="PSUM"))
            wqp = ctx.enter_context(tc.tile_pool(name="wqp", bufs=4))
            wop = ctx.enter_context(tc.tile_pool(name="wop", bufs=4))
            rp = ctx.enter_context(tc.tile_pool(name="rp", bufs=2))
            ep = ctx.enter_context(tc.tile_pool(name="ep", bufs=2))
            trp = ctx.enter_context(tc.tile_pool(name="trp", bufs=1))
            srp = ctx.enter_context(tc.tile_pool(name="srp", bufs=1))
            op = ctx.enter_context(tc.tile_pool(name="op", bufs=4))

            # identity arrives by DMA (cheaper than building it with
            # gpsimd memset + affine_select on the critical lead-in)
            ident_h = persist.tile([P, P], dt.float16, tag="ident_h")
            nc.sync.dma_start(ident_h[:], id_d[:])

            # PE warmup (HAM) while the first DMAs land; also preload the
            # Exp activation table.
            for i in range(48):
                warm = psA.tile([P, P], dt.float16, tag="sc", name=f"warm{i}")
                nc.tensor.transpose(warm[:], ident_h[:], ident_h[:])
            dummy = rp.tile([P, S], dt.float16, tag="ev", name="expwarm")
            nc.scalar.activation(dummy[:, 0:P], ident_h[:],
                                 mybir.ActivationFunctionType.Exp, scale=1.0)

            kt_t = persist.tile([P, KVPC, S], dt.float16, tag="kt")
            vnat = persist.tile([P, NTT, KVPC * HD], dt.float16, tag="vnat")
            qt = [persist.tile([P, S], dt.float16, tag=f"qa{h}", name=f"qt{h}")
                  for h in range(HPC)]

            def slab(w_dram, idx, nm):
                t = wqp.tile([P, NKT, P], dt.float16, tag="w", name=nm)
                nc.sync.dma_start(t[:], w_dram[:, idx])
                return t

            # DMA order on the sync queue matters: pass-1 slab chunks ride
            # with the x stream in k order (sv1 on the scalar engine's DMA
            # queue as a parallel path) so the pass-1 k-loop stays
            # compute-bound. First group in 2-k pieces so the very first
            # matmuls can start ~2us in.
            sk0 = wqp.tile([P, NKT, P], dt.float16, tag="w", name="sk0")
            sv0 = wqp.tile([P, NKT, P], dt.float16, tag="w", name="sv0")
            sk1 = wqp.tile([P, NKT, P], dt.float16, tag="w", name="sk1")
            sv1 = wqp.tile([P, NKT, P], dt.float16, tag="w", name="sv1")
            xT = persist.tile([P, NKT, S], dt.float16, tag="xT")
            groups = [slice(2 * i, 2 * (i + 1)) for i in range(4)] + \
                     [slice(8 * g, 8 * (g + 1)) for g in range(1, 4)]
            for ks in groups:
                nc.sync.dma_start(sk0[:, ks, :], wk_d[:, 0, ks, :])
                nc.scalar.dma_start(sv0[:, ks, :], wv_d[:, 0, ks, :])
                nc.scalar.dma_start(sv1[:, ks, :], wv_d[:, 1, ks, :])
                for k in range(ks.start, ks.stop):
                    nc.sync.dma_start(xT[:, k, :], xt_d[:, k, :])
            nc.sync.dma_start(sk1[:], wk_d[:, 1])
            # cos/sin ride the scalar engine's DMA queue behind the V slabs;
            # they are only needed at the pass-1 rope (~halfway in).
            cos2 = persist.tile([P, S], dt.float16, tag="cos2")
            nc.scalar.dma_start(cos2[:], cos_d[:])
            sinpm = persist.tile([P, S], dt.float16, tag="sinpm")
            nc.scalar.dma_start(sinpm[:], sin_d[:])

            def rope_evict(pq0, pq1, dest_ap):
                # psum fp32 -> fp16, then rotate-half rope in fp16 on DVE
                ev = rp.tile([P, S], dt.float16, tag="ev")
                nc.scalar.copy(ev[:, 0:HALF], pq0[:])
                nc.scalar.copy(ev[:, HALF:S], pq1[:])
                t1 = rp.tile([P, S], dt.float16, tag="t1")
                t2 = rp.tile([P, S], dt.float16, tag="t2")
                # sinsw rows: [0:64]=+sin (pairs ev re rows), [64:128]=-sin
                # (pairs ev im rows) so each DVE mul reads matching base
                # partitions (tensor_tensor SBUF inputs must share base).
                nc.vector.tensor_mul(out=t1[:], in0=ev[:], in1=cos2[:])
                nc.vector.tensor_mul(out=t2[0:64, :], in0=ev[64:P, :],
                                     in1=sinpm[64:P, :])
                nc.vector.tensor_mul(out=t2[64:P, :], in0=ev[0:64, :],
                                     in1=sinpm[0:64, :])
                nc.vector.tensor_add(out=dest_ap, in0=t1[:], in1=t2[:])

            # ---- Phase B pass 1: K0 (psB) + V0/V1 (psA), 6 MMs per k ----
            sq = slab(wq_d, 0, "sq0")
            pk0 = psB.tile([P, HALF], dt.float32, tag="pj", name="pk0")
            pk1 = psB.tile([P, HALF], dt.float32, tag="pj", name="pk1")
            pvA = psA.tile([P, S], dt.float32, tag="sc", name="pvA")
            pvB = psA.tile([P, S], dt.float32, tag="sc", name="pvB")
            for k in range(NKT):
                st, sp = (k == 0), (k == NKT - 1)
                x0, x1 = xT[:, k, 0:HALF], xT[:, k, HALF:S]
                nc.tensor.matmul(pk0[:], sk0[:, k], x0, start=st, stop=sp)
                nc.tensor.matmul(pk1[:], sk0[:, k], x1, start=st, stop=sp)
                nc.tensor.matmul(pvA[:, 0:HALF], sv0[:, k], x0,
                                 start=st, stop=sp, skip_group_check=True)
                nc.tensor.matmul(pvA[:, HALF:S], sv0[:, k], x1,
                                 start=st, stop=sp, skip_group_check=True)
                nc.tensor.matmul(pvB[:, 0:HALF], sv1[:, k], x0,
                                 start=st, stop=sp, skip_group_check=True)
                nc.tensor.matmul(pvB[:, HALF:S], sv1[:, k], x1,
                                 start=st, stop=sp, skip_group_check=True)
            # evictions free psB/psA for pass 2 while it runs
            rope_evict(pk0, pk1, kt_t[:, 0, :])
            vteA = rp.tile([P, S], dt.float16, tag="vt", name="vteA")
            nc.scalar.copy(vteA[:], pvA[:])
            vteB = rp.tile([P, S], dt.float16, tag="vt", name="vteB")
            nc.scalar.copy(vteB[:], pvB[:])

            # ---- Phase B pass 2: K1 (psC) staggered with Q0 (psB) ----
            pk2 = psC.tile([P, HALF], dt.float32, tag="po", name="pk2")
            pk3 = psC.tile([P, HALF], dt.float32, tag="po", name="pk3")
            pq0 = psB.tile([P, HALF], dt.float32, tag="pj", name="pq0_0")
            pq1 = psB.tile([P, HALF], dt.float32, tag="pj", name="pq1_0")

            def k1_step(k):
                st, sp = (k == 0), (k == NKT - 1)
                nc.tensor.matmul(pk2[:], sk1[:, k], xT[:, k, 0:HALF],
                                 start=st, stop=sp)
                nc.tensor.matmul(pk3[:], sk1[:, k], xT[:, k, HALF:S],
                                 start=st, stop=sp)

            def q0_step(k):
                st, sp = (k == 0), (k == NKT - 1)
                nc.tensor.matmul(pq0[:], sq[:, k], xT[:, k, 0:HALF],
                                 start=st, stop=sp)
                nc.tensor.matmul(pq1[:], sq[:, k], xT[:, k, HALF:S],
                                 start=st, stop=sp)

            for k in range(4):
                k1_step(k)
            for k in range(4, NKT):
                k1_step(k)
                q0_step(k - 4)
            for k in range(NKT - 4, NKT):
                q0_step(k)
            rope_evict(pq0, pq1, qt[0][:])
            rope_evict(pk2, pk3, kt_t[:, 1, :])
            for kv in range(KVPC):
                vte = vteA if kv == 0 else vteB
                for tt in range(NTT):
                    ptv = psA.tile([P, P], dt.float16, tag="sc",
                                   name=f"tv{kv}_{tt}")
                    nc.tensor.transpose(ptv[:], vte[:, tt * P:(tt + 1) * P],
                                        ident_h[:])
                    nc.vector.tensor_copy(vnat[:, tt, kv * HD:(kv + 1) * HD],
                                          ptv[:])
            sq = slab(wq_d, 1, "sq1")
            wos0 = wop.tile([P, HPC, P], dt.float16, tag="wo", name="wo0")
            nc.sync.dma_start(wos0[:], wo_d[:, 0])

            # ---- Stage 2: per-head attention interleaved with Q proj h+1 ----
            for h in range(HPC):
                kv = h // 4
                e = ep.tile([P, NTT, S], dt.float16, tag="e", name=f"e{h}")
                if h < HPC - 1:
                    pq0 = psB.tile([P, HALF], dt.float32, tag="pj",
                                   name=f"pq0_{h+1}")
                    pq1 = psB.tile([P, HALF], dt.float32, tag="pj",
                                   name=f"pq1_{h+1}")
                acc = None
                po0 = psC.tile([P, HALF], dt.float32, tag="po", name=f"po0_{h}")
                po1 = psC.tile([P, HALF], dt.float32, tag="po", name=f"po1_{h}")
                last = (h == HPC - 1)
                if last:
                    # the last head has no projection filler, so pre-run the
                    # head-0..6 accumulation of phase D's first three
                    # out-tiles inside its window (ot0 in-loop, ot1/ot2 after)
                    wos1 = wop.tile([P, HPC, P], dt.float16, tag="wo",
                                    name="wo1")
                    nc.sync.dma_start(wos1[:], wo_d[:, 1])
                    wos2 = wop.tile([P, HPC, P], dt.float16, tag="wo",
                                    name="wo2")
                    nc.sync.dma_start(wos2[:], wo_d[:, 2])
                    pd00 = psB.tile([P, HALF], dt.float32, tag="pj",
                                    name="pd0_0")
                    pd01 = psB.tile([P, HALF], dt.float32, tag="pj",
                                    name="pd1_0")

                def pv_step(tt):
                    vtile = vnat[:, tt, kv * HD:(kv + 1) * HD]
                    st, sp = (tt == 0), (tt == NTT - 1)
                    nc.tensor.matmul(po0[:], vtile, e[:, tt, 0:HALF],
                                     start=st, stop=sp)
                    nc.tensor.matmul(po1[:], vtile, e[:, tt, HALF:S],
                                     start=st, stop=sp)

                for tt in range(NTT):
                    sc = psA.tile([P, S], dt.float32, tag="sc",
                                  name=f"sc{h}_{tt}")
                    ktile = kt_t[:, kv, tt * P:(tt + 1) * P]
                    nc.tensor.matmul(sc[:, 0:HALF], ktile, qt[h][:, 0:HALF],
                                     start=True, stop=True)
                    nc.tensor.matmul(sc[:, HALF:S], ktile, qt[h][:, HALF:S],
                                     start=True, stop=True)
                    nc.scalar.activation(e[:, tt, :], sc[:],
                                         mybir.ActivationFunctionType.Exp,
                                         scale=float(SCALE))
                    if not last:
                        for j in range(4):
                            k = tt * 4 + j
                            st, sp = (k == 0), (k == NKT - 1)
                            nc.tensor.matmul(pq0[:], sq[:, k], xT[:, k, 0:HALF],
                                             start=st, stop=sp)
                            nc.tensor.matmul(pq1[:], sq[:, k], xT[:, k, HALF:S],
                                             start=st, stop=sp)
                    elif tt >= 1:
                        # no proj filler for the last head: stagger PV one
                        # tile behind scores and feed it phase D ot=0
                        # (heads 0..6 are ready) as extra PE work
                        pv_step(tt - 1)
                        dht = tt - 1
                        nc.tensor.matmul(pd00[:], wos0[:, dht, :],
                                         qt[dht][:, 0:HALF],
                                         start=(dht == 0), stop=False,
                                         skip_group_check=True)
                        nc.tensor.matmul(pd01[:], wos0[:, dht, :],
                                         qt[dht][:, HALF:S],
                                         start=(dht == 0), stop=False,
                                         skip_group_check=True)
                    # running softmax-denominator sum on DVE (fp16, 2x mode):
                    # serial accumulation leaves only one add after the
                    # final exp
                    if tt == 1:
                        acc = trp.tile([P, S], dt.float16, tag="s0",
                                       name=f"acc{h}")
                        nc.vector.tensor_add(out=acc[:], in0=e[:, 0, :],
                                             in1=e[:, 1, :])
                    elif tt >= 2:
                        nc.vector.tensor_add(out=acc[:], in0=acc[:],
                                             in1=e[:, tt, :])
                if not last:
                    rope_evict(pq0, pq1, qt[h + 1][:])
                if not last:
                    for tt in range(NTT):
                        pv_step(tt)
                else:
                    pv_step(NTT - 1)
                    pdA1 = psA.tile([P, S], dt.float32, tag="sc", name="pdA1")
                    pdA2 = psA.tile([P, S], dt.float32, tag="sc", name="pdA2")
                    for dht in range(HPC - 1):
                        for pdt, wot in ((pdA1, wos1), (pdA2, wos2)):
                            nc.tensor.matmul(pdt[:, 0:HALF], wot[:, dht, :],
                                             qt[dht][:, 0:HALF],
                                             start=(dht == 0), stop=False,
                                             skip_group_check=True)
                            nc.tensor.matmul(pdt[:, HALF:S], wot[:, dht, :],
                                             qt[dht][:, HALF:S],
                                             start=(dht == 0), stop=False,
                                             skip_group_check=True)
                # denominator: gpsimd all-reduce across partitions (one op
                # does reduce + broadcast, no PE matmuls, no psum), then
                # reciprocal; hides under the next head / the D prefill
                den = srp.tile([P, S], dt.float32, tag="rb", name=f"den{h}")
                nc.gpsimd.partition_all_reduce(den[:], acc[:], channels=P,
                                               reduce_op=bass_isa.ReduceOp.add)
                rcb = srp.tile([P, S], dt.float32, tag="rc", name=f"rcb{h}")
                nc.vector.reciprocal_approx_fast(rcb[:], den[:])
                attn = persist.tile([P, S], dt.float16, tag=f"qa{h}",
                                    name=f"attn{h}")
                nc.vector.tensor_mul(out=attn[:, 0:HALF], in0=po0[:],
                                     in1=rcb[:, 0:HALF])
                nc.vector.tensor_mul(out=attn[:, HALF:S], in0=po1[:],
                                     in1=rcb[:, HALF:S])
                qt[h] = attn
                if h < HPC - 2:
                    sq = slab(wq_d, h + 2, f"sq{h+2}")

            # ---- Phase D: out projection ----
            # ot 0..2 were pre-accumulated over heads 0..6 during the last
            # attention head; only their head-7 contribution remains.
            pre = {0: (pd00[:], pd01[:], wos0), 1: (pdA1[:, 0:HALF], pdA1[:, HALF:S], wos1),
                   2: (pdA2[:, 0:HALF], pdA2[:, HALF:S], wos2)}
            for ot in range(NOT):
                if ot < 3:
                    pd0, pd1, wosb = pre[ot]
                    ht = HPC - 1
                    nc.tensor.matmul(pd0, wosb[:, ht, :], qt[ht][:, 0:HALF],
                                     start=False, stop=True,
                                     skip_group_check=True)
                    nc.tensor.matmul(pd1, wosb[:, ht, :], qt[ht][:, HALF:S],
                                     start=False, stop=True,
                                     skip_group_check=True)
                else:
                    wosb = wop.tile([P, HPC, P], dt.float16, tag="wo",
                                    name=f"wo{ot}")
                    nc.sync.dma_start(wosb[:], wo_d[:, ot])
                    pool = psB if ot % 2 == 0 else psC
                    tag = "pj" if ot % 2 == 0 else "po"
                    pdt0 = pool.tile([P, HALF], dt.float32, tag=tag,
                                     name=f"pd0_{ot}")
                    pdt1 = pool.tile([P, HALF], dt.float32, tag=tag,
                                     name=f"pd1_{ot}")
                    pd0, pd1 = pdt0[:], pdt1[:]
                    for ht in range(HPC):
                        st, sp = (ht == 0), (ht == HPC - 1)
                        nc.tensor.matmul(pd0, wosb[:, ht, :], qt[ht][:, 0:HALF],
                                         start=st, stop=sp)
                        nc.tensor.matmul(pd1, wosb[:, ht, :], qt[ht][:, HALF:S],
                                         start=st, stop=sp)
                o0 = op.tile([P, HALF], dt.float16, tag="o", name=f"o0_{ot}")
                nc.scalar.copy(o0[:], pd0)
                nc.sync.dma_start(out_d[ot * P:(ot + 1) * P, 0:HALF], o0[:])
                o1 = op.tile([P, HALF], dt.float16, tag="o", name=f"o1_{ot}")
                nc.vector.tensor_copy(o1[:], pd1)
                nc.sync.dma_start(out_d[ot * P:(ot + 1) * P, HALF:S], o1[:])

    nc.compile()
    return nc


def _get_nc():
    if "nc" not in _CACHE:
        _CACHE["nc"] = _build()
    return _CACHE["nc"]


def _host_prep(x, freqs_cos, freqs_sin, wq, wk, wv, wo):
    x = np.asarray(x, dtype=np.float32)
    wq = np.asarray(wq, dtype=np.float32)
    wk = np.asarray(wk, dtype=np.float32)
    wv = np.asarray(wv, dtype=np.float32)
    wo = np.asarray(wo, dtype=np.float32)
    perm = np.empty(HD, np.int64)
    perm[0:64] = 2 * np.arange(64)
    perm[64:HD] = 2 * np.arange(64) + 1
    wqp = wq.reshape(DIM, N_HEADS, HD)[:, :, perm]
    wkp = wk.reshape(DIM, N_KV, HD)[:, :, perm]
    wvr = wv.reshape(DIM, N_KV, HD)
    cosT = np.asarray(freqs_cos, np.float32).T  # [64, S]
    sinT = np.asarray(freqs_sin, np.float32).T
    cos2 = np.ascontiguousarray(
        np.concatenate([cosT, cosT], axis=0)).astype(np.float16)   # [128, S]
    # swapped-half layout: rows 0..63 = +sin (multiplies ev re rows via
    # t2[64:128]), rows 64..127 = -sin (multiplies ev im rows via t2[0:64])
    sinpm = np.ascontiguousarray(
        np.concatenate([sinT, -sinT], axis=0)).astype(np.float16)

    def wslab(w_c, nh):
        # [DIM, nh, HD] -> [P, nh(out-tile), NKT, P]
        return np.ascontiguousarray(
            w_c.reshape(NKT, P, nh, HD).transpose(1, 2, 0, 3)).astype(np.float16)

    in_maps = []
    xt_b = {}
    for b in range(B):
        # [S, DIM] -> [P, NKT, S]
        xt_b[b] = np.ascontiguousarray(
            x[b].reshape(S, NKT, P).transpose(2, 1, 0)).astype(np.float16)
    for core in range(NCORES):
        b, g = core // 4, core % 4
        wo_c = wo[HPC * HD * g: HPC * HD * (g + 1), :]  # [1024, DIM]
        wo_slab = np.ascontiguousarray(
            wo_c.reshape(HPC, P, NOT, P).transpose(1, 2, 0, 3)).astype(np.float16)
        in_maps.append({
            "identh": np.eye(P, dtype=np.float16),
            "xt": xt_b[b],
            "wq": wslab(wqp[:, HPC * g: HPC * (g + 1), :], HPC),
            "wk": wslab(wkp[:, KVPC * g: KVPC * (g + 1), :], KVPC),
            "wv": wslab(wvr[:, KVPC * g: KVPC * (g + 1), :], KVPC),
            "wo": wo_slab,
            "cos2": cos2,
            "sinpm": sinpm,
        })
    return in_maps


def kernel(x, freqs_cos, freqs_sin, mask, input_indexes, wq, wk, wv, wo,
           cache_k, cache_v, **_ignored):
    in_maps = _host_prep(x, freqs_cos, freqs_sin, wq, wk, wv, wo)
    nc = _get_nc()
    res = run_bass_kernel_spmd(nc, in_maps, core_ids=list(range(NCORES)))
    outs = [res.results[c]["out"] for c in range(NCORES)]
    out = np.empty((B, S, DIM), np.float32)
    for b in range(B):
        acc = outs[4 * b].astype(np.float32)
        for g in range(1, 4):
            acc = acc + outs[4 * b + g].astype(np.float32)
        out[b] = acc.T
    return out
